# revision 18
# baseline (speedup 1.0000x reference)
"""Chamfer loss kernel for Trainium2 (8 NeuronCores, data-parallel over batch).

Math: for each sample b,
    loss[b] = sum_n o_w[b,n] * min_m(masked d2) + sum_m t_w[b,m] * min_n(masked d2)
with d2 the squared-distance matrix between outputs[b] ([N,D]) and targets[b]
([M,D]); masked entries excluded from the mins.  (The reference squares the
min *distance*, which equals the min squared distance.)

Design (v4):
  * Host compacts each sample to its live rows/cols (weights are {0,1}) and
    builds K=18 augmented fp16 operands so one matmul computes
    e[n,m] = 2*a.b - |a|^2 - |b|^2 = -d2 (single fp16; rel-err budget 2e-2).
    Padded rows/cols carry a -/+HUGE norm channel so they never win a max.
  * Per sample, 9 row tiles e_i [128n x m_pad] fp32 in PSUM (pairs of tiles
    on disjoint PE row groups via partition offsets 0/64).
  * ScalarE is the only PSUM consumer: casts each tile to fp16 SBUF (X_i).
    PSUM double-buffering then suffices; everything downstream runs on
    deeply-buffered SBUF, decoupling the engines.
  * VectorE (2x fp16 mode): per tile a half-fold into rf[:, i, :] (row-max
    stream) and a running colacc = max(colacc, X_i) (col-max stream).  After
    the 9 tiles: fold tree rf 544->272->136->68, final 1x reduce -> rowmax
    [128, 9]; PE transposes colacc blocks into PSUM fp16 and two 1x reduces
    give colmax [128, 9].
  * All per-(sample,tile) maxes land in one fp32 accumulator xd[128, S, 18];
    a single end-phase computes relu(-x) . live-mask, a free-axis reduce and
    a ones-matmul partition sum -> y[1, S].
"""

import math
import os

import numpy as np

NCORES = 8
HUGE = 30000.0   # pad-norm magnitude; -2*HUGE stays finite in fp16
K = 18

_PROGRAM_CACHE = {}


def _build_program(nt, mt, m_pad, n_samples):
    import concourse.bacc as bacc
    import concourse.mybir as mybir
    from concourse import tile

    f16 = mybir.dt.float16
    f32 = mybir.dt.float32
    Alu = mybir.AluOpType
    Act = mybir.ActivationFunctionType
    Axis = mybir.AxisListType

    n_pad = nt * 128
    S = n_samples
    NT = nt + mt                       # accumulator planes per sample
    h0 = m_pad // 2
    mblocks = [128] * (m_pad // 128)
    if m_pad % 128:
        mblocks.append(m_pad % 128)

    nc = bacc.Bacc("TRN2", target_bir_lowering=False, debug=False,
                   num_devices=NCORES)

    a_in = nc.dram_tensor("a_aug", [S, K, n_pad], f16, kind="ExternalInput")
    b_in = nc.dram_tensor("b_aug", [S, K, m_pad], f16, kind="ExternalInput")
    w_in = nc.dram_tensor("w", [S, 128, NT], f32, kind="ExternalInput")
    i_in = nc.dram_tensor("ident", [128, 128], f16, kind="ExternalInput")
    o_in = nc.dram_tensor("ones", [128, 1], f32, kind="ExternalInput")
    y_out = nc.dram_tensor("y", [1, S], f32, kind="ExternalOutput")

    with tile.TileContext(nc) as tc:
        with (
            tc.tile_pool(name="const", bufs=1) as constp,
            tc.tile_pool(name="ab", bufs=2) as abp,
            tc.tile_pool(name="x", bufs=5) as xp,
            tc.tile_pool(name="rf", bufs=2) as rfp,
            tc.tile_pool(name="ca", bufs=2) as cap,
            tc.tile_pool(name="fin", bufs=1) as finp,
            tc.tile_pool(name="ps", bufs=2, space="PSUM") as psp,
            tc.tile_pool(name="pst", bufs=2, space="PSUM") as pstp,
        ):
            ident = constp.tile([128, 128], f16)
            ones = constp.tile([128, 1], f32)
            wt = constp.tile([128, S, NT], f32)
            xd = constp.tile([128, S * NT], f32)
            out_sb = constp.tile([1, S], f32)

            for s in range(S):
                ah = abp.tile([64 + K, n_pad], f16, tag="ah")
                bh = abp.tile([64 + K, m_pad], f16, tag="bh")
                if s == 0:
                    # Critical-path operands first on the HW sync queue; the
                    # row-group duplicates ride the gpsimd SWDGE queue (only
                    # needed from the 3rd tile on); constants go behind the
                    # scalar queue's boot preamble (needed much later).
                    nc.sync.dma_start(ah[0:K, :], a_in[s, :, :])
                    nc.sync.dma_start(bh[0:K, :], b_in[s, :, :])
                    nc.gpsimd.dma_start(ah[64:64 + K, :], a_in[s, :, :])
                    nc.gpsimd.dma_start(bh[64:64 + K, :], b_in[s, :, :])
                    nc.scalar.dma_start(wt[:, s, :], w_in[s, :, :])
                    nc.scalar.dma_start(ident[:], i_in[:, :])
                    nc.scalar.dma_start(ones[:], o_in[:, :])
                else:
                    nc.sync.dma_start(ah[0:K, :], a_in[s, :, :])
                    nc.sync.dma_start(bh[0:K, :], b_in[s, :, :])
                    nc.sync.dma_start(ah[64:64 + K, :], a_in[s, :, :])
                    nc.sync.dma_start(bh[64:64 + K, :], b_in[s, :, :])
                    nc.sync.dma_start(wt[:, s, :], w_in[s, :, :])

                rf = rfp.tile([128, nt, h0], f16, tag="rf")
                colacc = cap.tile([128, m_pad], f16, tag="ca")
                xtiles = {}

                # Emit matmuls in pairs on disjoint PE row-groups; tile 0
                # solo (groups alternate per chunk) so the cast pipeline
                # starts immediately.
                if s == 0:
                    groups = [[0], [1]] + \
                        [[i for i in (i0, i0 + 1) if i < nt]
                         for i0 in range(2, nt, 2)]
                else:
                    groups = [[0]] + [[i for i in (i0, i0 + 1) if i < nt]
                                      for i0 in range(1, nt, 2)]
                for grp in groups:
                    pstile = {}
                    for i in grp:
                        ps = psp.tile([128, m_pad], f32, tag="ps")
                        pstile[i] = ps
                        off = 0
                        c = 0
                        while off < m_pad:
                            wc = min(512, m_pad - off)
                            if s == 0 and i < 2:
                                po = 0  # duplicates may not have landed yet
                            elif len(grp) > 1:
                                po = 64 * (i % 2)
                            else:
                                po = 64 * (c % 2)
                            nc.tensor.matmul(
                                ps[:, off:off + wc],
                                ah[po:po + K, i * 128:(i + 1) * 128],
                                bh[po:po + K, off:off + wc],
                                start=True, stop=True)
                            off += wc
                            c += 1
                    for i in grp:
                        x = xp.tile([128, m_pad], f16, tag="x")
                        xtiles[i] = x
                        nc.scalar.activation(x[:], pstile[i][:], Act.Copy)
                    for i in grp:
                        x = xtiles[i]
                        nc.vector.tensor_tensor(
                            rf[:, i, :], x[:, 0:h0], x[:, h0:m_pad],
                            op=Alu.max)
                        if i == 1:
                            nc.vector.tensor_tensor(
                                colacc[:], xtiles[0][:], x[:], op=Alu.max)
                        elif i > 1:
                            nc.vector.tensor_tensor(
                                colacc[:], colacc[:], x[:], op=Alu.max)

                # Row stream: fold tree along m, then one small 1x reduce.
                w = h0
                while w > 25 and w % 2 == 0:
                    h = w // 2
                    nc.vector.tensor_tensor(
                        rf[:, :, 0:h], rf[:, :, 0:h], rf[:, :, h:w],
                        op=Alu.max)
                    w = h
                nc.vector.tensor_reduce(
                    xd[:, s * NT:s * NT + nt], rf[:, :, 0:w],
                    axis=Axis.X, op=Alu.max)

                # Col stream: PE-transpose colacc blocks, reduce along the
                # (transposed) n axis.  Full-128 blocks grouped 8 per PSUM
                # bank; a partial tail block gets its own group so the
                # reduce only touches partitions its transpose wrote.
                nfull = len([b for b in mblocks if b == 128])
                grps = [mblocks[:nfull][i:i + 8]
                        for i in range(0, nfull, 8)]
                if nfull < len(mblocks):
                    grps.append([mblocks[-1]])
                bi = 0
                off = 0
                for g in grps:
                    bw0 = g[-1]  # only the last block can be partial
                    pst = pstp.tile([128, 1024], f16, tag="pst")
                    goff = 0
                    for k, bw in enumerate(g):
                        nc.tensor.transpose(
                            pst[0:bw, k * 128:(k + 1) * 128],
                            colacc[:, off + goff:off + goff + bw],
                            ident[:])
                        goff += bw
                    pst3 = pst[0:bw0, 0:len(g) * 128].rearrange(
                        "p (g q) -> p g q", q=128)
                    nc.vector.tensor_reduce(
                        xd[0:bw0, s * NT + nt + bi:s * NT + nt + bi + len(g)],
                        pst3, axis=Axis.X, op=Alu.max)
                    bi += len(g)
                    off += goff

            # ---- finalization (all samples) ----
            f1 = finp.tile([128, S * NT], f32, tag="f1")
            g1 = finp.tile([128, S, NT], f32, tag="g1")
            r1 = finp.tile([128, S], f32, tag="r1")
            nc.gpsimd.tensor_scalar(f1[:], xd[:], -1.0, 0.0,
                                    op0=Alu.mult, op1=Alu.max)
            f13 = f1[:].rearrange("p (s j) -> p s j", j=NT)
            nc.gpsimd.tensor_tensor(g1[:], f13, wt[:], op=Alu.mult)
            nc.vector.tensor_reduce(r1[:], g1[:], axis=Axis.X, op=Alu.add)
            pss = pstp.tile([1, S], f32, tag="pst")
            nc.tensor.matmul(pss[:], ones[:], r1[:], start=True, stop=True)
            nc.scalar.activation(out_sb[:], pss[:], Act.Copy)
            nc.sync.dma_start(y_out[:, :], out_sb[:])

    nc.compile()
    return nc


def _prep_sample(a_live, b_live, n_pad, m_pad):
    """Augmented operands: (A [18, n_pad], B [18, m_pad]) fp16 so that
    (A.T @ B)[n, m] = 2*a.b - |a|^2 - |b|^2 = -d2, padding pushed to -HUGE."""
    n_live, d = a_live.shape
    m_live = b_live.shape[0]
    assert d == 16

    a2 = np.sum(a_live.astype(np.float64) ** 2, axis=1)
    b2 = np.sum(b_live.astype(np.float64) ** 2, axis=1)

    A = np.zeros((K, n_pad), np.float16)
    A[0:16, :n_live] = (2.0 * a_live).astype(np.float16).T
    A[16, :] = np.float16(-1)
    A[17, :n_live] = (-a2).astype(np.float16)
    A[17, n_live:] = np.float16(-HUGE)

    B = np.zeros((K, m_pad), np.float16)
    B[0:16, :m_live] = b_live.astype(np.float16).T
    B[16, :m_live] = b2.astype(np.float16)
    B[16, m_live:] = np.float16(HUGE)
    B[17, :] = np.float16(1)
    return A, B


def kernel(o_weights, outputs, t_weights, targets):
    from concourse.bass_utils import run_bass_kernel_spmd

    o_weights = np.asarray(o_weights, np.float32)
    t_weights = np.asarray(t_weights, np.float32)
    outputs = np.asarray(outputs, np.float32)
    targets = np.asarray(targets, np.float32)

    B, N, D = outputs.shape
    M = targets.shape[1]
    assert B % NCORES == 0, f"batch {B} not divisible by {NCORES}"
    n_samples = B // NCORES

    o_idx = [np.nonzero(o_weights[b])[0] for b in range(B)]
    t_idx = [np.nonzero(t_weights[b])[0] for b in range(B)]
    max_n = max(1, max(len(ix) for ix in o_idx))
    max_m = max(1, max(len(ix) for ix in t_idx))
    nt = math.ceil(max_n / 128)
    n_pad = nt * 128
    m_pad = 64 * math.ceil(max_m / 64)
    mt = math.ceil(m_pad / 128)
    NT = nt + mt

    key = (nt, mt, m_pad, n_samples)
    if key not in _PROGRAM_CACHE:
        _PROGRAM_CACHE[key] = _build_program(nt, mt, m_pad, n_samples)
    nc = _PROGRAM_CACHE[key]

    a_aug = np.zeros((B, K, n_pad), np.float16)
    b_aug = np.zeros((B, K, m_pad), np.float16)
    w_arr = np.zeros((B, 128, NT), np.float32)
    for b in range(B):
        n_live, m_live = len(o_idx[b]), len(t_idx[b])
        a_aug[b], b_aug[b] = _prep_sample(
            outputs[b][o_idx[b]], targets[b][t_idx[b]], n_pad, m_pad)
        w_arr[b, :, 0:nt] = (np.arange(n_pad) < n_live).reshape(nt, 128).T
        mm = np.zeros(mt * 128, bool)
        mm[:m_pad] = np.arange(m_pad) < m_live
        w_arr[b, :, nt:NT] = mm.reshape(mt, 128).T

    ident = np.eye(128, dtype=np.float16)
    ones = np.ones((128, 1), np.float32)
    in_maps = []
    for k in range(NCORES):
        sl = slice(k * n_samples, (k + 1) * n_samples)
        in_maps.append({
            "a_aug": a_aug[sl], "b_aug": b_aug[sl], "w": w_arr[sl],
            "ident": ident, "ones": ones,
        })

    trace = bool(os.environ.get("CHAMFER_TRACE"))
    kw = {}
    if trace:
        kw = {"trace": True,
              "tmpdir": os.environ.get("CHAMFER_TRACE_DIR") or None}
    res = run_bass_kernel_spmd(nc, in_maps, list(range(NCORES)), **kw)
    if trace and res.exec_time_ns is not None:
        print(f"HW exec time: {res.exec_time_ns} ns")

    out = np.empty((B,), np.float32)
    for k in range(NCORES):
        out[k * n_samples:(k + 1) * n_samples] = res.results[k]["y"][0]
    return out


# revision 19
# speedup vs baseline: 1.0136x; 1.0136x over previous
"""Chamfer loss kernel for Trainium2 (8 NeuronCores, data-parallel over batch).

Math: for each sample b,
    loss[b] = sum_n o_w[b,n] * min_m(masked d2) + sum_m t_w[b,m] * min_n(masked d2)
with d2 the squared-distance matrix between outputs[b] ([N,D]) and targets[b]
([M,D]); masked entries excluded from the mins.  (The reference squares the
min *distance*, which equals the min squared distance.)

Design (v4):
  * Host compacts each sample to its live rows/cols (weights are {0,1}) and
    builds K=18 augmented fp16 operands so one matmul computes
    e[n,m] = 2*a.b - |a|^2 - |b|^2 = -d2 (single fp16; rel-err budget 2e-2).
    Padded rows/cols carry a -/+HUGE norm channel so they never win a max.
  * Per sample, 9 row tiles e_i [128n x m_pad] fp32 in PSUM (pairs of tiles
    on disjoint PE row groups via partition offsets 0/64).
  * ScalarE is the only PSUM consumer: casts each tile to fp16 SBUF (X_i).
    PSUM double-buffering then suffices; everything downstream runs on
    deeply-buffered SBUF, decoupling the engines.
  * VectorE (2x fp16 mode): per tile a half-fold into rf[:, i, :] (row-max
    stream) and a running colacc = max(colacc, X_i) (col-max stream).  After
    the 9 tiles: fold tree rf 544->272->136->68, final 1x reduce -> rowmax
    [128, 9]; PE transposes colacc blocks into PSUM fp16 and two 1x reduces
    give colmax [128, 9].
  * All per-(sample,tile) maxes land in one fp32 accumulator xd[128, S, 18];
    a single end-phase computes relu(-x) . live-mask, a free-axis reduce and
    a ones-matmul partition sum -> y[1, S].
"""

import math
import os

import numpy as np

NCORES = 8
HUGE = 30000.0   # pad-norm magnitude; -2*HUGE stays finite in fp16
K = 18

_PROGRAM_CACHE = {}


def _build_program(nt, mt, m_pad, n_samples):
    import concourse.bacc as bacc
    import concourse.mybir as mybir
    from concourse import tile

    f16 = mybir.dt.float16
    f32 = mybir.dt.float32
    Alu = mybir.AluOpType
    Act = mybir.ActivationFunctionType
    Axis = mybir.AxisListType

    n_pad = nt * 128
    S = n_samples
    NT = nt + mt                       # accumulator planes per sample
    h0 = m_pad // 2
    mblocks = [128] * (m_pad // 128)
    if m_pad % 128:
        mblocks.append(m_pad % 128)

    nc = bacc.Bacc("TRN2", target_bir_lowering=False, debug=False,
                   num_devices=NCORES)

    a_in = nc.dram_tensor("a_aug", [S, K, n_pad], f16, kind="ExternalInput")
    b_in = nc.dram_tensor("b_aug", [S, K, m_pad], f16, kind="ExternalInput")
    w_in = nc.dram_tensor("w", [S, 128, NT], f32, kind="ExternalInput")
    i_in = nc.dram_tensor("ident", [128, 128], f16, kind="ExternalInput")
    o_in = nc.dram_tensor("ones", [128, 1], f32, kind="ExternalInput")
    y_out = nc.dram_tensor("y", [1, S], f32, kind="ExternalOutput")

    with tile.TileContext(nc) as tc:
        with (
            tc.tile_pool(name="const", bufs=1) as constp,
            tc.tile_pool(name="ab", bufs=3) as abp,
            tc.tile_pool(name="x", bufs=5) as xp,
            tc.tile_pool(name="rf", bufs=2) as rfp,
            tc.tile_pool(name="ca", bufs=2) as cap,
            tc.tile_pool(name="fin", bufs=1) as finp,
            tc.tile_pool(name="ps", bufs=2, space="PSUM") as psp,
            tc.tile_pool(name="pst", bufs=2, space="PSUM") as pstp,
        ):
            ident = constp.tile([128, 128], f16)
            ones = constp.tile([128, 1], f32)
            wt = constp.tile([128, S, NT], f32)
            xd = constp.tile([128, S * NT], f32)
            out_sb = constp.tile([1, S], f32)

            for s in range(S):
                ah = abp.tile([64 + K, n_pad], f16, tag="ah")
                bh = abp.tile([64 + K, m_pad], f16, tag="bh")
                if s == 0:
                    # Critical-path operands first on the HW sync queue; the
                    # row-group duplicates ride the gpsimd SWDGE queue (only
                    # needed from the 3rd tile on); constants go behind the
                    # scalar queue's boot preamble (needed much later).
                    nc.sync.dma_start(ah[0:K, :], a_in[s, :, :])
                    nc.sync.dma_start(bh[0:K, :], b_in[s, :, :])
                    nc.gpsimd.dma_start(ah[64:64 + K, :], a_in[s, :, :])
                    nc.gpsimd.dma_start(bh[64:64 + K, :], b_in[s, :, :])
                    nc.scalar.dma_start(wt[:, s, :], w_in[s, :, :])
                    nc.scalar.dma_start(ident[:], i_in[:, :])
                    nc.scalar.dma_start(ones[:], o_in[:, :])
                else:
                    nc.sync.dma_start(ah[0:K, :], a_in[s, :, :])
                    nc.sync.dma_start(bh[0:K, :], b_in[s, :, :])
                    nc.sync.dma_start(ah[64:64 + K, :], a_in[s, :, :])
                    nc.sync.dma_start(bh[64:64 + K, :], b_in[s, :, :])
                    nc.scalar.dma_start(wt[:, s, :], w_in[s, :, :])

                rf = rfp.tile([128, nt, h0], f16, tag="rf")
                colacc = cap.tile([128, m_pad], f16, tag="ca")
                xtiles = {}

                # Emit matmuls in pairs on disjoint PE row-groups; tile 0
                # solo (groups alternate per chunk) so the cast pipeline
                # starts immediately.
                if s == 0:
                    groups = [[0], [1]] + \
                        [[i for i in (i0, i0 + 1) if i < nt]
                         for i0 in range(2, nt, 2)]
                else:
                    groups = [[0]] + [[i for i in (i0, i0 + 1) if i < nt]
                                      for i0 in range(1, nt, 2)]
                for grp in groups:
                    pstile = {}
                    for i in grp:
                        ps = psp.tile([128, m_pad], f32, tag="ps")
                        pstile[i] = ps
                        off = 0
                        c = 0
                        while off < m_pad:
                            wc = min(512, m_pad - off)
                            if s == 0 and i < 2:
                                po = 0  # duplicates may not have landed yet
                            elif len(grp) > 1:
                                po = 64 * (i % 2)
                            else:
                                po = 64 * (c % 2)
                            nc.tensor.matmul(
                                ps[:, off:off + wc],
                                ah[po:po + K, i * 128:(i + 1) * 128],
                                bh[po:po + K, off:off + wc],
                                start=True, stop=True)
                            off += wc
                            c += 1
                    for i in grp:
                        x = xp.tile([128, m_pad], f16, tag="x")
                        xtiles[i] = x
                        nc.scalar.activation(x[:], pstile[i][:], Act.Copy)
                    for i in grp:
                        x = xtiles[i]
                        nc.vector.tensor_tensor(
                            rf[:, i, :], x[:, 0:h0], x[:, h0:m_pad],
                            op=Alu.max)
                        if i == 1:
                            nc.vector.tensor_tensor(
                                colacc[:], xtiles[0][:], x[:], op=Alu.max)
                        elif i > 1:
                            nc.vector.tensor_tensor(
                                colacc[:], colacc[:], x[:], op=Alu.max)

                # Row stream: fold tree along m, then one small 1x reduce.
                w = h0
                while w > 25 and w % 2 == 0:
                    h = w // 2
                    nc.vector.tensor_tensor(
                        rf[:, :, 0:h], rf[:, :, 0:h], rf[:, :, h:w],
                        op=Alu.max)
                    w = h
                nc.vector.tensor_reduce(
                    xd[:, s * NT:s * NT + nt], rf[:, :, 0:w],
                    axis=Axis.X, op=Alu.max)

                # Col stream: PE-transpose colacc blocks, reduce along the
                # (transposed) n axis.  Full-128 blocks grouped 8 per PSUM
                # bank; a partial tail block gets its own group so the
                # reduce only touches partitions its transpose wrote.
                nfull = len([b for b in mblocks if b == 128])
                grps = [mblocks[:nfull][i:i + 8]
                        for i in range(0, nfull, 8)]
                if nfull < len(mblocks):
                    grps.append([mblocks[-1]])
                bi = 0
                off = 0
                for g in grps:
                    bw0 = g[-1]  # only the last block can be partial
                    pst = pstp.tile([128, 1024], f16, tag="pst")
                    goff = 0
                    for k, bw in enumerate(g):
                        nc.tensor.transpose(
                            pst[0:bw, k * 128:(k + 1) * 128],
                            colacc[:, off + goff:off + goff + bw],
                            ident[:])
                        goff += bw
                    pst3 = pst[0:bw0, 0:len(g) * 128].rearrange(
                        "p (g q) -> p g q", q=128)
                    nc.vector.tensor_reduce(
                        xd[0:bw0, s * NT + nt + bi:s * NT + nt + bi + len(g)],
                        pst3, axis=Axis.X, op=Alu.max)
                    bi += len(g)
                    off += goff

            # ---- finalization (all samples) ----
            f1 = finp.tile([128, S * NT], f32, tag="f1")
            g1 = finp.tile([128, S, NT], f32, tag="g1")
            r1 = finp.tile([128, S], f32, tag="r1")
            nc.gpsimd.tensor_scalar(f1[:], xd[:], -1.0, 0.0,
                                    op0=Alu.mult, op1=Alu.max)
            f13 = f1[:].rearrange("p (s j) -> p s j", j=NT)
            nc.gpsimd.tensor_tensor(g1[:], f13, wt[:], op=Alu.mult)
            nc.vector.tensor_reduce(r1[:], g1[:], axis=Axis.X, op=Alu.add)
            pss = pstp.tile([1, S], f32, tag="pst")
            nc.tensor.matmul(pss[:], ones[:], r1[:], start=True, stop=True)
            nc.scalar.activation(out_sb[:], pss[:], Act.Copy)
            nc.sync.dma_start(y_out[:, :], out_sb[:])

    nc.compile()
    return nc


def _prep_sample(a_live, b_live, n_pad, m_pad):
    """Augmented operands: (A [18, n_pad], B [18, m_pad]) fp16 so that
    (A.T @ B)[n, m] = 2*a.b - |a|^2 - |b|^2 = -d2, padding pushed to -HUGE."""
    n_live, d = a_live.shape
    m_live = b_live.shape[0]
    assert d == 16

    a2 = np.sum(a_live.astype(np.float64) ** 2, axis=1)
    b2 = np.sum(b_live.astype(np.float64) ** 2, axis=1)

    A = np.zeros((K, n_pad), np.float16)
    A[0:16, :n_live] = (2.0 * a_live).astype(np.float16).T
    A[16, :] = np.float16(-1)
    A[17, :n_live] = (-a2).astype(np.float16)
    A[17, n_live:] = np.float16(-HUGE)

    B = np.zeros((K, m_pad), np.float16)
    B[0:16, :m_live] = b_live.astype(np.float16).T
    B[16, :m_live] = b2.astype(np.float16)
    B[16, m_live:] = np.float16(HUGE)
    B[17, :] = np.float16(1)
    return A, B


def kernel(o_weights, outputs, t_weights, targets):
    from concourse.bass_utils import run_bass_kernel_spmd

    o_weights = np.asarray(o_weights, np.float32)
    t_weights = np.asarray(t_weights, np.float32)
    outputs = np.asarray(outputs, np.float32)
    targets = np.asarray(targets, np.float32)

    B, N, D = outputs.shape
    M = targets.shape[1]
    assert B % NCORES == 0, f"batch {B} not divisible by {NCORES}"
    n_samples = B // NCORES

    o_idx = [np.nonzero(o_weights[b])[0] for b in range(B)]
    t_idx = [np.nonzero(t_weights[b])[0] for b in range(B)]
    max_n = max(1, max(len(ix) for ix in o_idx))
    max_m = max(1, max(len(ix) for ix in t_idx))
    nt = math.ceil(max_n / 128)
    n_pad = nt * 128
    m_pad = 64 * math.ceil(max_m / 64)
    mt = math.ceil(m_pad / 128)
    NT = nt + mt

    key = (nt, mt, m_pad, n_samples)
    if key not in _PROGRAM_CACHE:
        _PROGRAM_CACHE[key] = _build_program(nt, mt, m_pad, n_samples)
    nc = _PROGRAM_CACHE[key]

    a_aug = np.zeros((B, K, n_pad), np.float16)
    b_aug = np.zeros((B, K, m_pad), np.float16)
    w_arr = np.zeros((B, 128, NT), np.float32)
    for b in range(B):
        n_live, m_live = len(o_idx[b]), len(t_idx[b])
        a_aug[b], b_aug[b] = _prep_sample(
            outputs[b][o_idx[b]], targets[b][t_idx[b]], n_pad, m_pad)
        w_arr[b, :, 0:nt] = (np.arange(n_pad) < n_live).reshape(nt, 128).T
        mm = np.zeros(mt * 128, bool)
        mm[:m_pad] = np.arange(m_pad) < m_live
        w_arr[b, :, nt:NT] = mm.reshape(mt, 128).T

    ident = np.eye(128, dtype=np.float16)
    ones = np.ones((128, 1), np.float32)
    in_maps = []
    for k in range(NCORES):
        sl = slice(k * n_samples, (k + 1) * n_samples)
        in_maps.append({
            "a_aug": a_aug[sl], "b_aug": b_aug[sl], "w": w_arr[sl],
            "ident": ident, "ones": ones,
        })

    trace = bool(os.environ.get("CHAMFER_TRACE"))
    kw = {}
    if trace:
        kw = {"trace": True,
              "tmpdir": os.environ.get("CHAMFER_TRACE_DIR") or None}
    res = run_bass_kernel_spmd(nc, in_maps, list(range(NCORES)), **kw)
    if trace and res.exec_time_ns is not None:
        print(f"HW exec time: {res.exec_time_ns} ns")

    out = np.empty((B,), np.float32)
    for k in range(NCORES):
        out[k * n_samples:(k + 1) * n_samples] = res.results[k]["y"][0]
    return out
